# revision 1
# baseline (speedup 1.0000x reference)
"""Trainium2 Bass kernel for nn_Encoder_82910048682485 (binary-tree GNN encoder).

Structure exploited: in the heap-layout complete binary tree, the children of
the contiguous parent range [2^l-1, 2^(l+1)-1) are exactly the contiguous
range [2^(l+1)-1, 2^(l+2)-1), and parent p's children are cols 2s / 2s+1 of
that block.  So the whole computation is a chain of matmuls over shrinking
contiguous blocks — no real gather/scatter.

Sharding: data-parallel over the 8 subtrees rooted at nodes 7..14 (level 3).
Each core owns 2^15 leaves and computes its subtree's 2^16-1 node embeddings.
The top 7 nodes (levels 0..2) are computed on host (7 rows of a 256->128 MLP,
~0.001% of FLOPs).

On-chip layout is transposed: embeddings are stored [EMB=128 partitions,
nodes as free dim].  Then the even/odd child split needed by the cell MLP is
just a stride-2 free-dim access pattern, and each level-up step is 6 PE
matmuls + 2 leaky-relu passes.  Leaf chunks stream in and a binary-counter
cascade of per-level pending buffers fuses all levels in SBUF (each node
embedding is written to HBM exactly once, read back never).

Matmul operands are fp16 by default (fp32r runs in the PE's half-duty
fp32-HIGH mode and never warms the HAM clock gate; fp16 streams 1 row/cycle
at 2.4 GHz like bf16 but keeps 10 mantissa bits, and halves the output DMA
bytes).  PSUM accumulation stays fp32.
When all biases are zero (true for this model), leaky-relu work is split
between the Scalar engine (native Lrelu) and the idle Vector engine
(0.01*x + 0.99*relu(x), two fused ALU ops), and the two halves of the hidden
layer share one [128, w] PSUM tile so one pass covers both.
"""

import sys

for _p in ("/opt/trn_rl_repo",):
    if _p not in sys.path:
        sys.path.insert(0, _p)

import numpy as np

import concourse.bacc as bacc
import concourse.bass as bass
import concourse.mybir as mybir
from concourse import tile
from concourse.bass_utils import run_bass_kernel_spmd

DEPTH = 18
EMB = 128
HID = 256
VAL = 32
N_LEAVES = 2 ** DEPTH
N_NODES = 2 ** (DEPTH + 1) - 1
N_CORES = 8
SUB = DEPTH - 3              # per-core subtree: levels 0..SUB, 2^SUB leaves
ALPHA = 0.01                 # jax.nn.leaky_relu default negative_slope

F32 = mybir.dt.float32
F32R = mybir.dt.float32r
BF16 = mybir.dt.bfloat16
FP16 = mybir.dt.float16
LRELU = mybir.ActivationFunctionType.Lrelu

# wpack column layout ([128, WPACK_COLS], matmul dtype):
_W1A = 0          # W1[0:128, :]    cols 0:256
_W1B = 256        # W1[128:256, :]  cols 256:512
_W2A = 512        # W2[0:128, :]    cols 512:640
_W2B = 640        # W2[128:256, :]  cols 640:768
_WE = 768         # We (rows 0:32)  cols 768:896
WPACK_COLS = 896
# bias tile columns ([128, 4] fp32): b1[0:128], b1[128:256], b2, be

# matmul operand precision (see module docstring); fp16 streams 1 row/cycle
# like bf16 but keeps 10 mantissa bits (values here are O(1), far from
# fp16 range limits)
MM_DT = FP16


def build_nc(sub=SUB, ch=1024, wcap=1024, n_lv_dmas=16,
             mm_dt=MM_DT, zero_bias=True, dve_out=True, dve_h_every=0,
             f32r_min_n=2):
    """Build the per-core SPMD Bass program.

    sub:        subtree leaf level (leaves = 2^sub)
    ch:         leaf chunk width (columns per leaf psum tile, <= 1024)
    wcap:       max pending-buffer width (columns consumed per upward step)
    zero_bias:  enables the fused single-pass h activation and DVE routing
                (only correct when b1 == 0)
    dve_out:    route the parent-output leaky-relu to the Vector engine
    dve_h_every: if k > 0, route every k-th h-activation to DVE as well
    """
    n_leaves = 2 ** sub
    n_out = 2 ** (sub + 1) - 1
    ch = min(ch, n_leaves)
    assert n_leaves % ch == 0
    n_chunks = n_leaves // ch
    assert n_chunks % n_lv_dmas == 0
    qs = n_leaves // n_lv_dmas          # leaf columns per input DMA
    chunks_per_q = n_chunks // n_lv_dmas

    def width(d):
        return min(wcap, 2 ** d)

    nc = bacc.Bacc("TRN2", target_bir_lowering=False, debug=False)
    lv_d = nc.dram_tensor("lvT", [VAL, n_leaves], mm_dt, kind="ExternalInput").ap()
    wp_d = nc.dram_tensor("wpack", [128, WPACK_COLS], mm_dt, kind="ExternalInput").ap()
    bias_d = nc.dram_tensor("bias", [128, 4], F32, kind="ExternalInput").ap()
    out_d = nc.dram_tensor("outT", [EMB, n_out], mm_dt, kind="ExternalOutput").ap()

    with tile.TileContext(nc) as tc:
        import contextlib
        with contextlib.ExitStack() as ctx:
            const_pool = ctx.enter_context(tc.tile_pool(name="const", bufs=1))
            lv_pool = ctx.enter_context(tc.tile_pool(name="lv", bufs=3))
            pend_pool = ctx.enter_context(tc.tile_pool(name="pend", bufs=4))
            hs_pool = ctx.enter_context(tc.tile_pool(name="hs", bufs=4))
            dvetmp_pool = ctx.enter_context(tc.tile_pool(name="dvetmp", bufs=4))
            # PSUM budget (8 banks): leaf [128,1024]x1 = 2, h [128,1024]x2 = 4,
            # o [128,512]x2 = 2.
            ps_leaf = ctx.enter_context(tc.tile_pool(name="psl", bufs=1, space="PSUM"))
            ps_h = ctx.enter_context(tc.tile_pool(name="psh", bufs=2, space="PSUM"))
            ps_o = ctx.enter_context(tc.tile_pool(name="pso", bufs=2, space="PSUM"))

            wp = const_pool.tile([128, WPACK_COLS], mm_dt, tag="wp")
            # We block first: it is all the leaf matmuls need, so the PE can
            # start ~5us earlier than waiting for the whole weight pack
            nc.sync.dma_start(wp[:, _WE:], wp_d[:, _WE:])
            bias = const_pool.tile([128, 4], F32, tag="bias")
            nc.sync.dma_start(bias[:], bias_d[:])
            nc.sync.dma_start(wp[:, 0:_WE], wp_d[:, 0:_WE])

            def act_lrelu(dst_ap, src_ap, bias_col):
                nc.scalar.activation(dst_ap, src_ap, LRELU,
                                     bias=bias[:, bias_col: bias_col + 1],
                                     alpha=ALPHA)

            def dve_lrelu(dst_ap, src_ap, w):
                # dst = 0.01*x + 0.99*relu(x)  (zero-bias leaky-relu; PSUM may
                # be read only once per instruction, hence the two-op form)
                tmp = dvetmp_pool.tile([128, w], F32, tag="dvetmp", name="dvetmp")
                nc.vector.tensor_scalar(tmp[:], src_ap, 0.0, 1.0 - ALPHA,
                                        mybir.AluOpType.max, mybir.AluOpType.mult)
                nc.vector.scalar_tensor_tensor(dst_ap, src_ap, float(ALPHA), tmp[:],
                                               mybir.AluOpType.mult,
                                               mybir.AluOpType.add)

            # per-level pending buffers (binary-counter cascade)
            cur_tile = {d: None for d in range(sub + 1)}
            cur_fill = {d: 0 for d in range(sub + 1)}
            base_col = {d: 0 for d in range(sub + 1)}
            consume_ctr = {"n": 0}

            def emit(d, w):
                """Reserve w columns at level d; returns (tile, offset)."""
                wd = width(d)
                if cur_tile[d] is None:
                    cur_tile[d] = pend_pool.tile([128, wd], mm_dt,
                                                 tag=f"p{d}", name=f"pend{d}")
                    cur_fill[d] = 0
                off = cur_fill[d]
                assert off + w <= wd
                cur_fill[d] = off + w
                return cur_tile[d], off

            def queue_full(d, j):
                """Detach level d's (full) pending tile and queue its consume."""
                assert cur_fill[d] == width(d)
                ready.append((d, cur_tile[d], cur_fill[d], j))
                cur_tile[d] = None
                cur_fill[d] = 0

            def consume(d, t, w, tail=False):
                """DMA a full level-d tile out and compute its parents into
                level d-1.  In the post-leaf-stream tail the leaf PSUM pool
                is idle; borrowing its slot every 3rd step deepens the
                h-pipeline from 2 to 3 and keeps the PE from cooling."""
                b = base_col[d]
                base_col[d] = b + w
                off0 = 2 ** d - 1
                nc.sync.dma_start(out_d[:, off0 + b: off0 + b + w], t[:, 0:w])
                if d == 0:
                    return
                consume_ctr["n"] += 1
                hw2 = w // 2
                E = t[:, 0:w:2]
                O = t[:, 1:w:2]
                # fp32r is ISA-illegal below a minimum moving size
                # (s3d3_mm_fp32r_restrictions); tiny steps fall back to fp32.
                cast = (lambda ap: ap) if (mm_dt != F32R or hw2 >= f32r_min_n) \
                    else (lambda ap: ap.bitcast(F32))
                if tail and consume_ctr["n"] % 3 == 2:
                    h = ps_leaf.tile([128, w], F32, tag="pl", name="h_tail")
                else:
                    h = ps_h.tile([128, w], F32, tag="h")
                nc.tensor.matmul(h[:, 0:hw2], cast(wp[:, 0:128]), cast(E),
                                 start=True, stop=False)
                nc.tensor.matmul(h[:, 0:hw2], cast(wp[:, _W1B: _W1B + 128]), cast(O),
                                 start=False, stop=True)
                nc.tensor.matmul(h[:, hw2:w], cast(wp[:, 128:256]), cast(E),
                                 start=True, stop=False)
                nc.tensor.matmul(h[:, hw2:w], cast(wp[:, _W1B + 128: _W1B + 256]),
                                 cast(O), start=False, stop=True)
                h_s = hs_pool.tile([128, w], mm_dt, tag="h_s")
                if zero_bias:
                    if dve_h_every and consume_ctr["n"] % dve_h_every == 0:
                        dve_lrelu(h_s[:], h[:], w)
                    else:
                        act_lrelu(h_s[:], h[:], 0)
                else:
                    act_lrelu(h_s[:, 0:hw2], h[:, 0:hw2], 0)
                    act_lrelu(h_s[:, hw2:w], h[:, hw2:w], 1)
                o_p = ps_o.tile([128, hw2], F32, tag="op")
                nc.tensor.matmul(o_p[:], cast(wp[:, _W2A: _W2A + 128]),
                                 cast(h_s[:, 0:hw2]), start=True, stop=False)
                nc.tensor.matmul(o_p[:], cast(wp[:, _W2B: _W2B + 128]),
                                 cast(h_s[:, hw2:w]), start=False, stop=True)
                dst, off = emit(d - 1, hw2)
                # DVE's two-op leaky-relu has ~2x the latency of ACT's native
                # one; small tiles sit on the serial tail chain of the
                # cascade, so only big mid-stream tiles go to DVE.
                if zero_bias and dve_out and hw2 >= 512:
                    dve_lrelu(dst[:, off: off + hw2], o_p[:], hw2)
                else:
                    act_lrelu(dst[:, off: off + hw2], o_p[:], 2)
                if cur_fill[d - 1] == width(d - 1):
                    queue_full(d - 1, cur_chunk["j"])

            # Deferred-consume queue: running a full cascade inline would put
            # a chain of dependent instructions at the head of the in-order
            # PE queue and stall it.  Instead, when a pending buffer fills it
            # is detached and queued, and one consume is drained per leaf
            # chunk — by then its inputs are a full chunk old, so the PE
            # never waits.
            ready = []
            cur_chunk = {"j": 0}

            def drain(n, min_age_chunk=None, tail=False):
                for _ in range(n):
                    if not ready:
                        return
                    if min_age_chunk is not None and ready[0][3] >= min_age_chunk:
                        return
                    dd, t, f, _j = ready.pop(0)
                    consume(dd, t, f, tail=tail)

            qt = None
            for j in range(n_chunks):
                if j % chunks_per_q == 0:
                    qt = lv_pool.tile([VAL, qs], mm_dt, tag="qt")
                    q = j // chunks_per_q
                    nc.sync.dma_start(qt[:], lv_d[:, q * qs: (q + 1) * qs])
                m = j % chunks_per_q
                p = ps_leaf.tile([128, ch], F32, tag="pl")
                for s in range(0, ch, 512):
                    sw = min(512, ch - s)
                    nc.tensor.matmul(p[:, s: s + sw], wp[0:32, _WE: _WE + 128],
                                     qt[:, m * ch + s: m * ch + s + sw],
                                     start=True, stop=True)
                dst, off = emit(sub, ch)
                act_lrelu(dst[:, off: off + ch], p[:], 3)
                if cur_fill[sub] == width(sub):
                    queue_full(sub, j)
                # drain up to 3 consumes whose inputs are at least one chunk
                # old (~2.3 consumes are generated per chunk; draining any
                # slower serializes the tree top in the tail)
                cur_chunk["j"] = j
                drain(3, min_age_chunk=j)
                # after cascade bursts, keep the backlog short so pending-
                # buffer slots recycle before the next fill needs them
                if len(ready) > 3:
                    drain(len(ready) - 3)
            while ready:
                drain(1, tail=True)

            assert all(cur_tile[d] is None for d in cur_tile), "unconsumed pending"
            assert all(base_col[d] == 2 ** d for d in base_col)

    # bacc passes: split multi-waits into event semaphores (HW allows one
    # sync wait per instruction), register allocation, DCE.
    nc.compile()
    return nc


def _leaky(v):
    return np.where(v >= 0, v, np.float32(ALPHA) * v).astype(np.float32)


def pack_weights(We, W1, W2):
    wpack = np.zeros((128, WPACK_COLS), np.float32)
    wpack[:, _W1A: _W1A + 256] = W1[0:128, :]
    wpack[:, _W1B: _W1B + 256] = W1[128:256, :]
    wpack[:, _W2A: _W2A + 128] = W2[0:128, :]
    wpack[:, _W2B: _W2B + 128] = W2[128:256, :]
    wpack[0:32, _WE: _WE + 128] = We
    return wpack


def pack_bias(b1, b2, be):
    bias = np.zeros((128, 4), np.float32)
    bias[:, 0] = b1[0:128]
    bias[:, 1] = b1[128:256]
    bias[:, 2] = b2
    bias[:, 3] = be
    return bias


def _np_dt(dt_):
    if dt_ == BF16:
        import ml_dtypes
        return ml_dtypes.bfloat16
    if dt_ == FP16:
        return np.float16
    return np.float32


_NC_CACHE = {}


def kernel(leaf_values, We, be, W1, b1, W2, b2, _trace=False):
    leaf_values = np.asarray(leaf_values, np.float32)
    We = np.asarray(We, np.float32)
    be = np.asarray(be, np.float32)
    W1 = np.asarray(W1, np.float32)
    b1 = np.asarray(b1, np.float32)
    W2 = np.asarray(W2, np.float32)
    b2 = np.asarray(b2, np.float32)

    sub_leaves = 2 ** SUB

    npdt = _np_dt(MM_DT)
    zero_bias = not b1.any()
    wpack = pack_weights(We, W1, W2).astype(npdt)
    bias = pack_bias(b1, b2, be)
    lvT = leaf_values.reshape(N_CORES, sub_leaves, VAL).transpose(0, 2, 1)
    in_maps = [
        {"lvT": np.ascontiguousarray(lvT[c]).astype(npdt), "wpack": wpack,
         "bias": bias}
        for c in range(N_CORES)
    ]

    key = (MM_DT, zero_bias)
    if _NC_CACHE.get("key") != key:
        _NC_CACHE["nc"] = build_nc(mm_dt=MM_DT, zero_bias=zero_bias)
        _NC_CACHE["key"] = key
    nc = _NC_CACHE["nc"]

    res = run_bass_kernel_spmd(nc, in_maps, list(range(N_CORES)), trace=_trace)
    outs = [np.asarray(res.results[c]["outT"], np.float32) for c in range(N_CORES)]

    embs = np.empty((N_NODES, EMB), np.float32)
    for c in range(N_CORES):
        full = np.ascontiguousarray(outs[c].T)        # [sub_nodes, 128]
        for d in range(SUB + 1):
            L = 3 + d
            n = 1 << d
            g0 = (1 << L) - 1 + c * n
            embs[g0: g0 + n] = full[n - 1: 2 * n - 1]

    # top 3 levels (nodes 0..6) on host
    lvl = np.stack([outs[c][:, 0] for c in range(N_CORES)])   # [8, 128]
    for l in (2, 1, 0):
        x = lvl.reshape(2 ** l, 2 * EMB)
        h = _leaky(x @ W1 + b1)
        lvl = _leaky(h @ W2 + b2)
        embs[(1 << l) - 1: (1 << (l + 1)) - 1] = lvl

    if _trace:
        kernel.last_results = res
    return embs



# revision 4
# speedup vs baseline: 1.0986x; 1.0986x over previous
"""Trainium2 Bass kernel for nn_Encoder_82910048682485 (binary-tree GNN encoder).

Structure exploited: in the heap-layout complete binary tree, the children of
the contiguous parent range [2^l-1, 2^(l+1)-1) are exactly the contiguous
range [2^(l+1)-1, 2^(l+2)-1), and parent p's children are cols 2s / 2s+1 of
that block.  So the whole computation is a chain of matmuls over shrinking
contiguous blocks — no real gather/scatter.

Sharding: data-parallel over the 8 subtrees rooted at nodes 7..14 (level 3).
Each core owns 2^15 leaves.  The on-chip layout is transposed: embeddings are
stored [EMB=128 partitions, nodes free].  Leaf chunks stream in and fused
per-level pending tiles cascade upward entirely in SBUF.

v2 changes over the first working version (181-217us):
 * o-layer runs as ONE fp8e4 DoubleRow matmul (256-deep contraction in a
   single pass) instead of two fp16 matmuls.  The hidden activations are
   written as fp8 by the very same PSUM->SBUF leaky-relu pass that was
   already needed, so the precision change costs no extra element work.
   Measured end-to-end fro error ~1.5e-2 (vs 3.4e-4 all-fp16) — inside the
   2e-2 gate; the h-layer and leaf embedder stay fp16.
 * The trace showed the PSUM->SBUF leaky-relu passes (ACT 68%, DVE 52%)
   rival the PE (68% union) as the wall.  Each job is now routed by a
   greedy balancer between native ACT lrelu and a 2-op DVE form.  (Pool
   cannot help: it has no PSUM port and supports no 2-tensor-input ops,
   and walrus rejects reading PSUM twice in one instruction.)
 * Consumes run in same-level PAIRS: both tiles' o DoubleRow matmuls land
   in one [128,1024] PSUM tile, so one activation instruction covers both
   (the ~290ns fixed ACT cost was 40% of a [128,512] job).  With l_stop=8
   every pair's output exactly fills the next level's tile, which also
   kills the partial-fill bookkeeping.
 * The serial tree top (per-core levels 0..7) moves to the host: those
   consumes are tiny but sit on a long dependency chain at the end.  The
   device writes levels 8..15; numpy finishes 255 nodes per core.
"""

import sys

for _p in ("/opt/trn_rl_repo",):
    if _p not in sys.path:
        sys.path.insert(0, _p)

import numpy as np

import concourse.bacc as bacc
import concourse.bass as bass
import concourse.mybir as mybir
from concourse import tile
from concourse.bass_utils import run_bass_kernel_spmd

DEPTH = 18
EMB = 128
HID = 256
VAL = 32
N_LEAVES = 2 ** DEPTH
N_NODES = 2 ** (DEPTH + 1) - 1
N_CORES = 8
SUB = DEPTH - 3              # per-core subtree: levels 0..SUB, 2^SUB leaves
L_STOP = 8                   # device computes levels SUB..L_STOP of the subtree
ALPHA = 0.01                 # jax.nn.leaky_relu default negative_slope

F32 = mybir.dt.float32
BF16 = mybir.dt.bfloat16
FP16 = mybir.dt.float16
FP8 = mybir.dt.float8e4
LRELU = mybir.ActivationFunctionType.Lrelu
DR = mybir.MatmulPerfMode.DoubleRow

# wp16 column layout ([128, WP16_COLS] fp16):
_W1A0 = 0        # W1[0:128, 0:128]
_W1B0 = 128      # W1[128:256, 0:128]
_W1A1 = 256      # W1[0:128, 128:256]
_W1B1 = 384      # W1[128:256, 128:256]
_W2A = 512       # W2[0:128, :]   (fp16 fallback / non-fp8 path)
_W2B = 640       # W2[128:256, :]
_WE = 768        # We (rows 0:32)
WP16_COLS = 896
# wp8: [128, 2, 128] fp8e4: [:,0,:]=W2[0:128,:], [:,1,:]=W2[128:256,:]
# bias tile columns ([128, 4] fp32): b1[0:128], b1[128:256], b2, be


class _Balancer:
    """Greedy router of PSUM->SBUF leaky-relu jobs over ACT / DVE.

    Costs are ns estimates from the measured HW trace: ACT ~(w+352)/1.2,
    DVE op ~1.04w+195.
    """

    def __init__(self, nc, scr_pool, use_dve=True):
        self.nc = nc
        self.scr = scr_pool
        self.use_dve = use_dve
        self.load = {"ACT": 0.0, "DVE": 0.0}
        self.n = {"ACT": 0, "DVE2": 0}

    def lrelu(self, dst_ap, src_ap, w, prefer=None):
        nc = self.nc
        c_act = 0.833 * w + 293
        c_dve2 = 2.08 * w + 390
        opts = [("ACT", max(self.load["ACT"] + c_act, self.load["DVE"]))]
        if self.use_dve:
            opts.append(("DVE2", max(self.load["ACT"],
                                     self.load["DVE"] + c_dve2)))
        route = prefer if prefer is not None else min(opts, key=lambda kv: kv[1])[0]
        self.n[route] = self.n.get(route, 0) + 1
        if route == "ACT":
            self.load["ACT"] += c_act
            nc.scalar.activation(dst_ap, src_ap, LRELU, alpha=ALPHA)
        else:
            self.load["DVE"] += c_dve2
            tmp = self.scr.tile([128, w], FP16, tag="scr", name="scr")
            nc.vector.tensor_scalar(tmp[:], src_ap, 0.0, 1.0 - ALPHA,
                                    mybir.AluOpType.max, mybir.AluOpType.mult)
            nc.vector.scalar_tensor_tensor(dst_ap, src_ap, float(ALPHA),
                                           tmp[:], mybir.AluOpType.mult,
                                           mybir.AluOpType.add)


def build_nc(sub=SUB, ch=1024, wcap=1024, n_lv_dmas=16, l_stop=L_STOP,
             zero_bias=True, o_fp8=True, use_dve=True,
             drain_per_chunk=2, backlog=4, min_age=1):
    """Build the per-core SPMD Bass program.

    sub:       subtree leaf level (leaves = 2^sub)
    l_stop:    lowest level computed on device (host does < l_stop)
    zero_bias: enables the DVE activation route (correct only when b==0)
    o_fp8:     o-layer as one fp8 DoubleRow matmul (else two fp16 matmuls)
    """
    n_leaves = 2 ** sub
    n_out = 2 ** (sub + 1) - 1
    ch = min(ch, n_leaves)
    assert n_leaves % ch == 0
    n_chunks = n_leaves // ch
    n_lv_dmas = min(n_lv_dmas, n_chunks)
    assert n_chunks % n_lv_dmas == 0
    qs = n_leaves // n_lv_dmas
    chunks_per_q = n_chunks // n_lv_dmas
    assert 0 <= l_stop < sub
    # pair-consume invariants: every level tile is exactly filled by its
    # producer (leaf chunk, pair-consume, or single consume)
    assert ch == wcap and 2 ** l_stop <= wcap

    def width(d):
        return min(wcap, 2 ** d)

    def n_tiles(d):
        return max(1, 2 ** d // wcap)

    nc = bacc.Bacc("TRN2", target_bir_lowering=False, debug=False)
    lv_d = nc.dram_tensor("lvT", [VAL, n_leaves], FP16, kind="ExternalInput").ap()
    wp16_d = nc.dram_tensor("wp16", [128, WP16_COLS], FP16,
                            kind="ExternalInput").ap()
    wp8_d = nc.dram_tensor("wp8", [128, 2, 128], FP8, kind="ExternalInput").ap()
    bias_d = nc.dram_tensor("bias", [128, 4], F32, kind="ExternalInput").ap()
    out_d = nc.dram_tensor("outT", [EMB, n_out], FP16, kind="ExternalOutput").ap()

    with tile.TileContext(nc) as tc:
        import contextlib
        with contextlib.ExitStack() as ctx:
            const_pool = ctx.enter_context(tc.tile_pool(name="const", bufs=1))
            lv_pool = ctx.enter_context(tc.tile_pool(name="lv", bufs=3))
            pend_pool = ctx.enter_context(tc.tile_pool(name="pend", bufs=6))
            h8_pool = ctx.enter_context(tc.tile_pool(name="h8", bufs=4))
            scr_pool = ctx.enter_context(tc.tile_pool(name="scr", bufs=4))
            # PSUM budget (8 banks): leaf [128,1024] = 2, h [128,1024]x2 = 4,
            # o [128,1024]x1 = 2.
            ps_leaf = ctx.enter_context(tc.tile_pool(name="psl", bufs=1, space="PSUM"))
            ps_h = ctx.enter_context(tc.tile_pool(name="psh", bufs=2, space="PSUM"))
            ps_o = ctx.enter_context(tc.tile_pool(name="pso", bufs=1, space="PSUM"))

            wp = const_pool.tile([128, WP16_COLS], FP16, tag="wp")
            # We block first: it is all the leaf matmuls need
            nc.sync.dma_start(wp[:, _WE:], wp16_d[:, _WE:])
            wp8 = const_pool.tile([128, 2, 128], FP8, tag="wp8")
            nc.sync.dma_start(wp8[:], wp8_d)
            bias = const_pool.tile([128, 4], F32, tag="bias")
            if not zero_bias:
                nc.sync.dma_start(bias[:], bias_d[:])
            nc.sync.dma_start(wp[:, 0:_WE], wp16_d[:, 0:_WE])

            bal = _Balancer(nc, scr_pool, use_dve=use_dve and zero_bias)

            def act_lrelu(dst_ap, src_ap, bias_col):
                # bias path (generality; real model has all-zero biases)
                nc.scalar.activation(dst_ap, src_ap, LRELU,
                                     bias=bias[:, bias_col: bias_col + 1],
                                     alpha=ALPHA)

            def h_lrelu(h8_ap, h_ap, w, hw2):
                if zero_bias:
                    bal.lrelu(h8_ap, h_ap, w)
                else:
                    # split so each half gets its own bias column
                    act_lrelu(h8_ap[:, 0:hw2], h_ap[:, 0:hw2], 0)
                    act_lrelu(h8_ap[:, hw2:w], h_ap[:, hw2:w], 1)

            def o_lrelu(dst_ap, src_ap, w):
                if zero_bias:
                    bal.lrelu(dst_ap, src_ap, w)
                else:
                    act_lrelu(dst_ap, src_ap, 2)

            base_col = {d: 0 for d in range(l_stop, sub + 1)}
            ready = {d: [] for d in range(l_stop, sub + 1)}  # (tile, birth_j)
            done_tiles = {d: 0 for d in range(l_stop, sub + 1)}
            cur_chunk = {"j": 0}

            def dma_out(d, t, w):
                b = base_col[d]
                base_col[d] = b + w
                off0 = 2 ** d - 1
                nc.sync.dma_start(out_d[:, off0 + b: off0 + b + w], t[:, 0:w])

            def deliver(d, t):
                """A freshly produced full tile for level d."""
                done_tiles[d] += 1
                if d == l_stop:
                    dma_out(d, t, width(d))
                else:
                    ready[d].append((t, cur_chunk["j"]))

            def cell(t, w, o_ap):
                """Children tile t [128,w] -> parent pre-acts into o_ap."""
                hw2 = w // 2
                E = t[:, 0:w:2]
                O = t[:, 1:w:2]
                h = ps_h.tile([128, w], F32, tag="h")
                nc.tensor.matmul(h[:, 0:hw2], wp[:, _W1A0:_W1A0 + 128], E,
                                 start=True, stop=False)
                nc.tensor.matmul(h[:, 0:hw2], wp[:, _W1B0:_W1B0 + 128], O,
                                 start=False, stop=True)
                nc.tensor.matmul(h[:, hw2:w], wp[:, _W1A1:_W1A1 + 128], E,
                                 start=True, stop=False)
                nc.tensor.matmul(h[:, hw2:w], wp[:, _W1B1:_W1B1 + 128], O,
                                 start=False, stop=True)
                if o_fp8:
                    h8 = h8_pool.tile([128, w], FP8, tag="h8")
                    h_lrelu(h8[:], h[:], w, hw2)
                    rhs3 = h8[:].rearrange("p (two n) -> p two n", two=2)
                    nc.tensor.matmul(o_ap, wp8[:], rhs3,
                                     start=True, stop=True, perf_mode=DR)
                else:
                    h_s = h8_pool.tile([128, w], FP16, tag="h8")
                    h_lrelu(h_s[:], h[:], w, hw2)
                    nc.tensor.matmul(o_ap, wp[:, _W2A:_W2A + 128],
                                     h_s[:, 0:hw2], start=True, stop=False)
                    nc.tensor.matmul(o_ap, wp[:, _W2B:_W2B + 128],
                                     h_s[:, hw2:w], start=False, stop=True)

            def consume_pair(d, tA, tB):
                w = width(d)
                dma_out(d, tA, w)
                dma_out(d, tB, w)
                o_p = ps_o.tile([128, w], F32, tag="op")
                cell(tA, w, o_p[:, 0: w // 2])
                cell(tB, w, o_p[:, w // 2: w])
                dst = pend_pool.tile([128, w], FP16, tag=f"p{d - 1}",
                                     name=f"pend{d - 1}")
                o_lrelu(dst[:], o_p[:], w)
                deliver(d - 1, dst)

            def consume_single(d, t):
                w = width(d)
                hw2 = w // 2
                dma_out(d, t, w)
                o_p = ps_o.tile([128, hw2], F32, tag="op")
                cell(t, w, o_p[:])
                dst = pend_pool.tile([128, hw2], FP16, tag=f"p{d - 1}",
                                     name=f"pend{d - 1}")
                o_lrelu(dst[:], o_p[:], hw2)
                deliver(d - 1, dst)

            def drain(budget, min_age_chunk=None):
                """Consume up to `budget` pair-equivalents of ready tiles."""
                j = cur_chunk["j"]
                progress = True
                while budget > 0 and progress:
                    progress = False
                    for d in range(sub, l_stop, -1):
                        q = ready[d]
                        if not q:
                            continue
                        if min_age_chunk is not None and q[0][1] >= min_age_chunk:
                            continue
                        if n_tiles(d) == 1:
                            t, _ = q.pop(0)
                            consume_single(d, t)
                            budget -= 1
                            progress = True
                        elif len(q) >= 2:
                            if min_age_chunk is not None and q[1][1] >= min_age_chunk:
                                continue
                            (tA, _), (tB, _) = q.pop(0), q.pop(0)
                            consume_pair(d, tA, tB)
                            budget -= 1
                            progress = True
                        if budget <= 0:
                            break

            def backlog_size():
                return sum(len(q) for q in ready.values())

            qt = None
            for j in range(n_chunks):
                if j % chunks_per_q == 0:
                    qt = lv_pool.tile([VAL, qs], FP16, tag="qt")
                    q = j // chunks_per_q
                    nc.sync.dma_start(qt[:], lv_d[:, q * qs: (q + 1) * qs])
                m = j % chunks_per_q
                p = ps_leaf.tile([128, ch], F32, tag="pl")
                for s in range(0, ch, 512):
                    sw = min(512, ch - s)
                    nc.tensor.matmul(p[:, s: s + sw], wp[0:32, _WE: _WE + 128],
                                     qt[:, m * ch + s: m * ch + s + sw],
                                     start=True, stop=True)
                dst = pend_pool.tile([128, ch], FP16, tag=f"p{sub}",
                                     name=f"pend{sub}")
                if zero_bias:
                    bal.lrelu(dst[:], p[:], ch)
                else:
                    act_lrelu(dst[:], p[:], 3)
                cur_chunk["j"] = j
                deliver(sub, dst)
                drain(drain_per_chunk, min_age_chunk=j - min_age + 1)
                if backlog_size() > backlog:
                    drain(backlog_size() - backlog)
            while backlog_size():
                drain(1)

            for d in range(l_stop, sub + 1):
                assert done_tiles[d] == n_tiles(d), (d, done_tiles[d])
                assert base_col[d] == 2 ** d, (d, base_col[d])

    nc.compile()
    nc._bal_stats = dict(bal.n)
    return nc


def _leaky(v):
    return np.where(v >= 0, v, np.float32(ALPHA) * v).astype(np.float32)


def pack_wp16(We, W1, W2):
    wp16 = np.zeros((128, WP16_COLS), np.float32)
    wp16[:, _W1A0:_W1A0 + 128] = W1[0:128, 0:128]
    wp16[:, _W1B0:_W1B0 + 128] = W1[128:256, 0:128]
    wp16[:, _W1A1:_W1A1 + 128] = W1[0:128, 128:256]
    wp16[:, _W1B1:_W1B1 + 128] = W1[128:256, 128:256]
    wp16[:, _W2A:_W2A + 128] = W2[0:128, :]
    wp16[:, _W2B:_W2B + 128] = W2[128:256, :]
    wp16[0:32, _WE:_WE + 128] = We
    return wp16


def pack_wp8(W2):
    import ml_dtypes
    wp8 = np.zeros((128, 2, 128), np.float32)
    wp8[:, 0, :] = W2[0:128, :]
    wp8[:, 1, :] = W2[128:256, :]
    return wp8.astype(ml_dtypes.float8_e4m3)


def pack_bias(b1, b2, be):
    bias = np.zeros((128, 4), np.float32)
    bias[:, 0] = b1[0:128]
    bias[:, 1] = b1[128:256]
    bias[:, 2] = b2
    bias[:, 3] = be
    return bias


_NC_CACHE = {}


def kernel(leaf_values, We, be, W1, b1, W2, b2, _trace=False):
    leaf_values = np.asarray(leaf_values, np.float32)
    We = np.asarray(We, np.float32)
    be = np.asarray(be, np.float32)
    W1 = np.asarray(W1, np.float32)
    b1 = np.asarray(b1, np.float32)
    W2 = np.asarray(W2, np.float32)
    b2 = np.asarray(b2, np.float32)

    sub_leaves = 2 ** SUB
    zero_bias = not (b1.any() or b2.any() or be.any())

    wp16 = pack_wp16(We, W1, W2).astype(np.float16)
    wp8 = pack_wp8(W2)
    bias = pack_bias(b1, b2, be)
    lvT = leaf_values.reshape(N_CORES, sub_leaves, VAL).transpose(0, 2, 1)
    in_maps = [
        {"lvT": np.ascontiguousarray(lvT[c]).astype(np.float16),
         "wp16": wp16, "wp8": wp8, "bias": bias}
        for c in range(N_CORES)
    ]

    key = ("v2.1", zero_bias)
    if _NC_CACHE.get("key") != key:
        _NC_CACHE["nc"] = build_nc(zero_bias=zero_bias)
        _NC_CACHE["key"] = key
    nc = _NC_CACHE["nc"]

    res = run_bass_kernel_spmd(nc, in_maps, list(range(N_CORES)), trace=_trace)
    outs = [np.asarray(res.results[c]["outT"], np.float32) for c in range(N_CORES)]

    embs = np.empty((N_NODES, EMB), np.float32)
    for c in range(N_CORES):
        full = np.ascontiguousarray(outs[c].T)        # [sub_nodes, 128]
        for d in range(L_STOP, SUB + 1):
            L = 3 + d
            n = 1 << d
            g0 = (1 << L) - 1 + c * n
            embs[g0: g0 + n] = full[n - 1: 2 * n - 1]

    # per-core levels L_STOP-1..0 on host (255 nodes/core, <1% of FLOPs)
    roots = np.empty((N_CORES, EMB), np.float32)
    for c in range(N_CORES):
        n0 = 1 << L_STOP
        lvl = np.ascontiguousarray(outs[c][:, n0 - 1: 2 * n0 - 1].T)
        for d in range(L_STOP - 1, -1, -1):
            x = lvl.reshape(1 << d, 2 * EMB)
            h = _leaky(x @ W1 + b1)
            lvl = _leaky(h @ W2 + b2)
            L = 3 + d
            g0 = (1 << L) - 1 + c * (1 << d)
            embs[g0: g0 + (1 << d)] = lvl
        roots[c] = lvl[0]

    # top 3 levels (nodes 0..6) on host
    lvl = roots
    for l in (2, 1, 0):
        x = lvl.reshape(2 ** l, 2 * EMB)
        h = _leaky(x @ W1 + b1)
        lvl = _leaky(h @ W2 + b2)
        embs[(1 << l) - 1: (1 << (l + 1)) - 1] = lvl

    if _trace:
        kernel.last_results = res
    return embs


# revision 7
# speedup vs baseline: 1.1404x; 1.0380x over previous
"""Trainium2 Bass kernel for nn_Encoder_82910048682485 (binary-tree GNN encoder).

Structure exploited: in the heap-layout complete binary tree, the children of
the contiguous parent range [2^l-1, 2^(l+1)-1) are exactly the contiguous
range [2^(l+1)-1, 2^(l+2)-1), and parent p's children are cols 2s / 2s+1 of
that block.  So the whole computation is a chain of matmuls over shrinking
contiguous blocks — no real gather/scatter.

Sharding: data-parallel over the 8 subtrees rooted at nodes 7..14 (level 3).
Each core owns 2^15 leaves.  The on-chip layout is transposed: embeddings are
stored [EMB=128 partitions, nodes free].  Leaf chunks stream in and fused
per-level pending tiles cascade upward entirely in SBUF.

v2 changes over the first working version (181-217us):
 * o-layer runs as ONE fp8e4 DoubleRow matmul (256-deep contraction in a
   single pass) instead of two fp16 matmuls.  The hidden activations are
   written as fp8 by the very same PSUM->SBUF leaky-relu pass that was
   already needed, so the precision change costs no extra element work.
   Measured end-to-end fro error ~1.5e-2 (vs 3.4e-4 all-fp16) — inside the
   2e-2 gate; the h-layer and leaf embedder stay fp16.
 * The trace showed the PSUM->SBUF leaky-relu passes (ACT 68%, DVE 52%)
   rival the PE (68% union) as the wall.  Each job is now routed by a
   greedy balancer between native ACT lrelu and a 2-op DVE form.  (Pool
   cannot help: it has no PSUM port and supports no 2-tensor-input ops,
   and walrus rejects reading PSUM twice in one instruction.)
 * Consumes run in same-level PAIRS: both tiles' o DoubleRow matmuls land
   in one [128,1024] PSUM tile, so one activation instruction covers both
   (the ~290ns fixed ACT cost was 40% of a [128,512] job).  With l_stop=8
   every pair's output exactly fills the next level's tile, which also
   kills the partial-fill bookkeeping.
 * The serial tree top (per-core levels 0..7) moves to the host: those
   consumes are tiny but sit on a long dependency chain at the end.  The
   device writes levels 8..15; numpy finishes 255 nodes per core.
"""

import sys

for _p in ("/opt/trn_rl_repo",):
    if _p not in sys.path:
        sys.path.insert(0, _p)

import numpy as np

import concourse.bacc as bacc
import concourse.bass as bass
import concourse.mybir as mybir
from concourse import tile
from concourse.bass_utils import run_bass_kernel_spmd

DEPTH = 18
EMB = 128
HID = 256
VAL = 32
N_LEAVES = 2 ** DEPTH
N_NODES = 2 ** (DEPTH + 1) - 1
N_CORES = 8
SUB = DEPTH - 3              # per-core subtree: levels 0..SUB, 2^SUB leaves
L_STOP = 8                   # device computes levels SUB..L_STOP of the subtree
ALPHA = 0.01                 # jax.nn.leaky_relu default negative_slope

F32 = mybir.dt.float32
BF16 = mybir.dt.bfloat16
FP16 = mybir.dt.float16
FP8 = mybir.dt.float8e4
LRELU = mybir.ActivationFunctionType.Lrelu
DR = mybir.MatmulPerfMode.DoubleRow

# wp16 column layout ([128, WP16_COLS] fp16):
_W1A0 = 0        # W1[0:128, 0:128]
_W1B0 = 128      # W1[128:256, 0:128]
_W1A1 = 256      # W1[0:128, 128:256]
_W1B1 = 384      # W1[128:256, 128:256]
_W2A = 512       # W2[0:128, :]   (fp16 fallback / non-fp8 path)
_W2B = 640       # W2[128:256, :]
_WE = 768        # We (rows 0:32)
WP16_COLS = 896
# wp8: [128, 2, 128] fp8e4: [:,0,:]=W2[0:128,:], [:,1,:]=W2[128:256,:]
# bias tile columns ([128, 4] fp32): b1[0:128], b1[128:256], b2, be


class _Balancer:
    """Greedy router of PSUM->SBUF leaky-relu jobs over ACT / DVE.

    Costs are ns estimates from the measured HW trace: ACT ~(w+352)/1.2,
    DVE op ~1.04w+195.
    """

    def __init__(self, nc, scr_pool, use_dve=True):
        self.nc = nc
        self.scr = scr_pool
        self.use_dve = use_dve
        self.load = {"ACT": 0.0, "DVE": 0.0}
        self.n = {"ACT": 0, "DVE2": 0}

    def lrelu(self, dst_ap, src_ap, w, prefer=None):
        nc = self.nc
        c_act = 0.833 * w + 293
        c_dve2 = 2.08 * w + 390
        opts = [("ACT", max(self.load["ACT"] + c_act, self.load["DVE"]))]
        if self.use_dve:
            opts.append(("DVE2", max(self.load["ACT"],
                                     self.load["DVE"] + c_dve2)))
        route = prefer if prefer is not None else min(opts, key=lambda kv: kv[1])[0]
        self.n[route] = self.n.get(route, 0) + 1
        if route == "ACT":
            self.load["ACT"] += c_act
            nc.scalar.activation(dst_ap, src_ap, LRELU, alpha=ALPHA)
        else:
            self.load["DVE"] += c_dve2
            tmp = self.scr.tile([128, w], FP16, tag="scr", name="scr")
            nc.vector.tensor_scalar(tmp[:], src_ap, 0.0, 1.0 - ALPHA,
                                    mybir.AluOpType.max, mybir.AluOpType.mult)
            nc.vector.scalar_tensor_tensor(dst_ap, src_ap, float(ALPHA),
                                           tmp[:], mybir.AluOpType.mult,
                                           mybir.AluOpType.add)


def build_nc(sub=SUB, ch=1024, wcap=1024, n_lv_dmas=16, l_stop=L_STOP,
             zero_bias=True, o_fp8=True, use_dve=True,
             drain_per_chunk=2, backlog=4, min_age=1):
    """Build the per-core SPMD Bass program.

    sub:       subtree leaf level (leaves = 2^sub)
    l_stop:    lowest level computed on device (host does < l_stop)
    zero_bias: enables the DVE activation route (correct only when b==0)
    o_fp8:     o-layer as one fp8 DoubleRow matmul (else two fp16 matmuls)
    """
    n_leaves = 2 ** sub
    n_out = 2 ** (sub + 1) - 1
    ch = min(ch, n_leaves)
    assert n_leaves % ch == 0
    n_chunks = n_leaves // ch
    n_lv_dmas = min(n_lv_dmas, n_chunks)
    assert n_chunks % n_lv_dmas == 0
    qs = n_leaves // n_lv_dmas
    chunks_per_q = n_chunks // n_lv_dmas
    assert 0 <= l_stop < sub
    # pair-consume invariants: every level tile is exactly filled by its
    # producer (leaf chunk, pair-consume, or single consume)
    assert ch == wcap and 2 ** l_stop <= wcap

    def width(d):
        return min(wcap, 2 ** d)

    def n_tiles(d):
        return max(1, 2 ** d // wcap)

    nc = bacc.Bacc("TRN2", target_bir_lowering=False, debug=False)
    lv_d = nc.dram_tensor("lvT", [VAL, n_leaves], FP16, kind="ExternalInput").ap()
    wp16_d = nc.dram_tensor("wp16", [128, WP16_COLS], FP16,
                            kind="ExternalInput").ap()
    wp8_d = nc.dram_tensor("wp8", [128, 2, 128], FP8, kind="ExternalInput").ap()
    bias_d = nc.dram_tensor("bias", [128, 4], F32, kind="ExternalInput").ap()
    out_d = nc.dram_tensor("outT", [EMB, n_out], FP16, kind="ExternalOutput").ap()

    with tile.TileContext(nc) as tc:
        import contextlib
        with contextlib.ExitStack() as ctx:
            const_pool = ctx.enter_context(tc.tile_pool(name="const", bufs=1))
            lv_pool = ctx.enter_context(tc.tile_pool(name="lv", bufs=3))
            pend_pool = ctx.enter_context(tc.tile_pool(name="pend", bufs=6))
            h8_pool = ctx.enter_context(tc.tile_pool(name="h8", bufs=4))
            scr_pool = ctx.enter_context(tc.tile_pool(name="scr", bufs=4))
            # PSUM budget (8 banks): leaf [128,1024] = 2, h [128,1024]x2 = 4,
            # o [128,1024]x1 = 2.
            ps_leaf = ctx.enter_context(tc.tile_pool(name="psl", bufs=1, space="PSUM"))
            ps_h = ctx.enter_context(tc.tile_pool(name="psh", bufs=2, space="PSUM"))
            ps_o = ctx.enter_context(tc.tile_pool(name="pso", bufs=1, space="PSUM"))

            wp = const_pool.tile([128, WP16_COLS], FP16, tag="wp")
            # We block first: it is all the leaf matmuls need
            nc.sync.dma_start(wp[:, _WE:], wp16_d[:, _WE:])
            wp8 = const_pool.tile([128, 2, 128], FP8, tag="wp8")
            nc.sync.dma_start(wp8[:], wp8_d)
            bias = const_pool.tile([128, 4], F32, tag="bias")
            if not zero_bias:
                nc.sync.dma_start(bias[:], bias_d[:])
            nc.sync.dma_start(wp[:, 0:_WE], wp16_d[:, 0:_WE])

            bal = _Balancer(nc, scr_pool, use_dve=use_dve and zero_bias)

            def act_lrelu(dst_ap, src_ap, bias_col):
                # bias path (generality; real model has all-zero biases)
                nc.scalar.activation(dst_ap, src_ap, LRELU,
                                     bias=bias[:, bias_col: bias_col + 1],
                                     alpha=ALPHA)

            def h_lrelu(h8_ap, h_ap, w, hw2):
                if zero_bias:
                    bal.lrelu(h8_ap, h_ap, w)
                else:
                    # split so each half gets its own bias column
                    act_lrelu(h8_ap[:, 0:hw2], h_ap[:, 0:hw2], 0)
                    act_lrelu(h8_ap[:, hw2:w], h_ap[:, hw2:w], 1)

            def o_lrelu(dst_ap, src_ap, w):
                if zero_bias:
                    bal.lrelu(dst_ap, src_ap, w)
                else:
                    act_lrelu(dst_ap, src_ap, 2)

            base_col = {d: 0 for d in range(l_stop, sub + 1)}
            ready = {d: [] for d in range(l_stop, sub + 1)}  # (tile, birth_j)
            done_tiles = {d: 0 for d in range(l_stop, sub + 1)}
            cur_chunk = {"j": 0}

            def dma_out(d, t, w):
                b = base_col[d]
                base_col[d] = b + w
                off0 = 2 ** d - 1
                nc.sync.dma_start(out_d[:, off0 + b: off0 + b + w], t[:, 0:w])

            def deliver(d, t):
                """A freshly produced full tile for level d."""
                done_tiles[d] += 1
                if d == l_stop:
                    dma_out(d, t, width(d))
                else:
                    ready[d].append((t, cur_chunk["j"]))

            def h_stage(t, w):
                """Children tile -> hidden pre-acts in PSUM; returns h tile."""
                hw2 = w // 2
                E = t[:, 0:w:2]
                O = t[:, 1:w:2]
                h = ps_h.tile([128, w], F32, tag="h")
                nc.tensor.matmul(h[:, 0:hw2], wp[:, _W1A0:_W1A0 + 128], E,
                                 start=True, stop=False)
                nc.tensor.matmul(h[:, 0:hw2], wp[:, _W1B0:_W1B0 + 128], O,
                                 start=False, stop=True)
                nc.tensor.matmul(h[:, hw2:w], wp[:, _W1A1:_W1A1 + 128], E,
                                 start=True, stop=False)
                nc.tensor.matmul(h[:, hw2:w], wp[:, _W1B1:_W1B1 + 128], O,
                                 start=False, stop=True)
                return h

            def h_stage_pair(tA, tB, w):
                """h matmuls for both tiles, interleaved so consecutive
                matmuls share the stationary operand (half the weight-buffer
                churn; loads get a full matmul to hide under)."""
                hw2 = w // 2
                EA, OA = tA[:, 0:w:2], tA[:, 1:w:2]
                EB, OB = tB[:, 0:w:2], tB[:, 1:w:2]
                hA = ps_h.tile([128, w], F32, tag="h")
                hB = ps_h.tile([128, w], F32, tag="h")
                nc.tensor.matmul(hA[:, 0:hw2], wp[:, _W1A0:_W1A0 + 128], EA,
                                 start=True, stop=False)
                nc.tensor.matmul(hB[:, 0:hw2], wp[:, _W1A0:_W1A0 + 128], EB,
                                 start=True, stop=False)
                nc.tensor.matmul(hA[:, 0:hw2], wp[:, _W1B0:_W1B0 + 128], OA,
                                 start=False, stop=True)
                nc.tensor.matmul(hB[:, 0:hw2], wp[:, _W1B0:_W1B0 + 128], OB,
                                 start=False, stop=True)
                nc.tensor.matmul(hA[:, hw2:w], wp[:, _W1A1:_W1A1 + 128], EA,
                                 start=True, stop=False)
                nc.tensor.matmul(hB[:, hw2:w], wp[:, _W1A1:_W1A1 + 128], EB,
                                 start=True, stop=False)
                nc.tensor.matmul(hA[:, hw2:w], wp[:, _W1B1:_W1B1 + 128], OA,
                                 start=False, stop=True)
                nc.tensor.matmul(hB[:, hw2:w], wp[:, _W1B1:_W1B1 + 128], OB,
                                 start=False, stop=True)
                return hA, hB

            def act_stage(h, w):
                """PSUM h -> SBUF activated (fp8 when o_fp8)."""
                hw2 = w // 2
                dt_ = FP8 if o_fp8 else FP16
                h8 = h8_pool.tile([128, w], dt_, tag="h8")
                h_lrelu(h8[:], h[:], w, hw2)
                return h8

            def o_stage(h8, w, o_ap):
                hw2 = w // 2
                if o_fp8:
                    rhs3 = h8[:].rearrange("p (two n) -> p two n", two=2)
                    nc.tensor.matmul(o_ap, wp8[:], rhs3,
                                     start=True, stop=True, perf_mode=DR)
                else:
                    nc.tensor.matmul(o_ap, wp[:, _W2A:_W2A + 128],
                                     h8[:, 0:hw2], start=True, stop=False)
                    nc.tensor.matmul(o_ap, wp[:, _W2B:_W2B + 128],
                                     h8[:, hw2:w], start=False, stop=True)

            def phase1(d, tiles):
                """DMA the consumed tiles out, run h matmuls + activations.
                Returns a staged record for phase2."""
                w = width(d)
                for t in tiles:
                    dma_out(d, t, w)
                if len(tiles) == 2:
                    hA, hB = h_stage_pair(tiles[0], tiles[1], w)
                    h8s = [act_stage(hA, w), act_stage(hB, w)]
                else:
                    h8s = [act_stage(h_stage(tiles[0], w), w)]
                return (d, w, h8s)

            def phase2(rec):
                """o matmuls into one PSUM tile + one output activation."""
                d, w, h8s = rec
                hw2 = w // 2
                ow = hw2 * len(h8s)
                o_p = ps_o.tile([128, ow], F32, tag="op")
                for k, h8 in enumerate(h8s):
                    o_stage(h8, w, o_p[:, k * hw2: (k + 1) * hw2])
                dst = pend_pool.tile([128, ow], FP16, tag=f"p{d - 1}",
                                     name=f"pend{d - 1}")
                o_lrelu(dst[:], o_p[:], ow)
                deliver(d - 1, dst)

            staged = []

            def pop_ready(min_age_chunk):
                for d in range(sub, l_stop, -1):
                    q = ready[d]
                    if not q:
                        continue
                    if min_age_chunk is not None and q[0][1] >= min_age_chunk:
                        continue
                    if n_tiles(d) == 1:
                        t, _ = q.pop(0)
                        return (d, [t])
                    if len(q) >= 2:
                        if min_age_chunk is not None and q[1][1] >= min_age_chunk:
                            continue
                        (tA, _), (tB, _) = q.pop(0), q.pop(0)
                        return (d, [tA, tB])
                return None

            def drain(budget, min_age_chunk=None):
                """Process up to `budget` pipeline units.  Each unit issues
                phase1 of the next ready tile-group, then phase2 of the
                previously staged group — so a group's o-matmuls enter the
                in-order PE queue a full unit after its h-activations were
                queued, and never stall the PE."""
                while budget > 0:
                    nxt = pop_ready(min_age_chunk)
                    if nxt is None and not staged:
                        return
                    if nxt is not None:
                        rec = phase1(nxt[0], nxt[1])
                        if staged:
                            phase2(staged.pop(0))
                        staged.append(rec)
                    else:
                        phase2(staged.pop(0))
                    budget -= 1

            def backlog_size():
                return sum(len(q) for q in ready.values()) + len(staged)

            qt = None
            for j in range(n_chunks):
                if j % chunks_per_q == 0:
                    qt = lv_pool.tile([VAL, qs], FP16, tag="qt")
                    q = j // chunks_per_q
                    nc.sync.dma_start(qt[:], lv_d[:, q * qs: (q + 1) * qs])
                m = j % chunks_per_q
                p = ps_leaf.tile([128, ch], F32, tag="pl")
                for s in range(0, ch, 512):
                    sw = min(512, ch - s)
                    nc.tensor.matmul(p[:, s: s + sw], wp[0:32, _WE: _WE + 128],
                                     qt[:, m * ch + s: m * ch + s + sw],
                                     start=True, stop=True)
                dst = pend_pool.tile([128, ch], FP16, tag=f"p{sub}",
                                     name=f"pend{sub}")
                if zero_bias:
                    bal.lrelu(dst[:], p[:], ch)
                else:
                    act_lrelu(dst[:], p[:], 3)
                cur_chunk["j"] = j
                deliver(sub, dst)
                drain(drain_per_chunk, min_age_chunk=j - min_age + 1)
                if backlog_size() > backlog:
                    drain(backlog_size() - backlog)
            while backlog_size():
                drain(1)

            for d in range(l_stop, sub + 1):
                assert done_tiles[d] == n_tiles(d), (d, done_tiles[d])
                assert base_col[d] == 2 ** d, (d, base_col[d])

    nc.compile()
    nc._bal_stats = dict(bal.n)
    return nc


def _leaky(v):
    return np.where(v >= 0, v, np.float32(ALPHA) * v).astype(np.float32)


def pack_wp16(We, W1, W2):
    wp16 = np.zeros((128, WP16_COLS), np.float32)
    wp16[:, _W1A0:_W1A0 + 128] = W1[0:128, 0:128]
    wp16[:, _W1B0:_W1B0 + 128] = W1[128:256, 0:128]
    wp16[:, _W1A1:_W1A1 + 128] = W1[0:128, 128:256]
    wp16[:, _W1B1:_W1B1 + 128] = W1[128:256, 128:256]
    wp16[:, _W2A:_W2A + 128] = W2[0:128, :]
    wp16[:, _W2B:_W2B + 128] = W2[128:256, :]
    wp16[0:32, _WE:_WE + 128] = We
    return wp16


def pack_wp8(W2):
    import ml_dtypes
    wp8 = np.zeros((128, 2, 128), np.float32)
    wp8[:, 0, :] = W2[0:128, :]
    wp8[:, 1, :] = W2[128:256, :]
    return wp8.astype(ml_dtypes.float8_e4m3)


def pack_bias(b1, b2, be):
    bias = np.zeros((128, 4), np.float32)
    bias[:, 0] = b1[0:128]
    bias[:, 1] = b1[128:256]
    bias[:, 2] = b2
    bias[:, 3] = be
    return bias


_NC_CACHE = {}


def kernel(leaf_values, We, be, W1, b1, W2, b2, _trace=False):
    leaf_values = np.asarray(leaf_values, np.float32)
    We = np.asarray(We, np.float32)
    be = np.asarray(be, np.float32)
    W1 = np.asarray(W1, np.float32)
    b1 = np.asarray(b1, np.float32)
    W2 = np.asarray(W2, np.float32)
    b2 = np.asarray(b2, np.float32)

    sub_leaves = 2 ** SUB
    zero_bias = not (b1.any() or b2.any() or be.any())

    wp16 = pack_wp16(We, W1, W2).astype(np.float16)
    wp8 = pack_wp8(W2)
    bias = pack_bias(b1, b2, be)
    lvT = leaf_values.reshape(N_CORES, sub_leaves, VAL).transpose(0, 2, 1)
    in_maps = [
        {"lvT": np.ascontiguousarray(lvT[c]).astype(np.float16),
         "wp16": wp16, "wp8": wp8, "bias": bias}
        for c in range(N_CORES)
    ]

    key = ("v2.1", zero_bias)
    if _NC_CACHE.get("key") != key:
        _NC_CACHE["nc"] = build_nc(zero_bias=zero_bias)
        _NC_CACHE["key"] = key
    nc = _NC_CACHE["nc"]

    res = run_bass_kernel_spmd(nc, in_maps, list(range(N_CORES)), trace=_trace)
    outs = [np.asarray(res.results[c]["outT"], np.float32) for c in range(N_CORES)]

    embs = np.empty((N_NODES, EMB), np.float32)
    for c in range(N_CORES):
        full = np.ascontiguousarray(outs[c].T)        # [sub_nodes, 128]
        for d in range(L_STOP, SUB + 1):
            L = 3 + d
            n = 1 << d
            g0 = (1 << L) - 1 + c * n
            embs[g0: g0 + n] = full[n - 1: 2 * n - 1]

    # per-core levels L_STOP-1..0 on host (255 nodes/core, <1% of FLOPs)
    roots = np.empty((N_CORES, EMB), np.float32)
    for c in range(N_CORES):
        n0 = 1 << L_STOP
        lvl = np.ascontiguousarray(outs[c][:, n0 - 1: 2 * n0 - 1].T)
        for d in range(L_STOP - 1, -1, -1):
            x = lvl.reshape(1 << d, 2 * EMB)
            h = _leaky(x @ W1 + b1)
            lvl = _leaky(h @ W2 + b2)
            L = 3 + d
            g0 = (1 << L) - 1 + c * (1 << d)
            embs[g0: g0 + (1 << d)] = lvl
        roots[c] = lvl[0]

    # top 3 levels (nodes 0..6) on host
    lvl = roots
    for l in (2, 1, 0):
        x = lvl.reshape(2 ** l, 2 * EMB)
        h = _leaky(x @ W1 + b1)
        lvl = _leaky(h @ W2 + b2)
        embs[(1 << l) - 1: (1 << (l + 1)) - 1] = lvl

    if _trace:
        kernel.last_results = res
    return embs
